# revision 1
# baseline (speedup 1.0000x reference)
"""Trainium2 Bass kernel for ControlLoRACrossAttnProcessor.

Head-parallel sharding over 8 NeuronCores: core c owns attention head c
(columns c*128:(c+1)*128 of Wq/Wk/Wv, rows of the same range in Wo's
contraction dim). Each core computes its head's full attention and a
partial output projection; the host sums the 8 partials. The rank-4
LoRA path is sharded by sequence rows (core c owns rows c*512:(c+1)*512)
and returned as a separate small output that the host adds in, together
with the output bias (added exactly once per row).

All matmuls run as float32r (TF32-like PE mode, full rate at moving
dim >= 256) with fp32 accumulation in PSUM. Attention uses the
transposed-scores layout: scores^T tiles [k=128, q=512] so softmax's
exp rides ScalarE and the k-sums ride TensorE (ones-vector matmul);
normalization is deferred to the output projection (divide commutes
with the linear Wo projection).
"""

import sys
import types

for _p in ("/opt/trn_rl_repo", "/root/.axon_site"):
    if _p not in sys.path:
        sys.path.insert(0, _p)

import numpy as np

import concourse.bass as bass  # noqa: E402
import concourse.mybir as mybir  # noqa: E402
from concourse import bacc  # noqa: E402
from concourse.bass_utils import run_bass_kernel_spmd  # noqa: E402
from concourse.tile import TileContext  # noqa: E402
from concourse.masks import make_identity  # noqa: E402

dt = mybir.dt

B, S, D = 2, 2048, 1024
H = 8
HD = 128
RANK = 4
N_CORES = 8
SG = B * S            # 4096 flattened rows
ROWS_PER_CORE = SG // N_CORES  # 512
NSTRIP = S // 512     # 4 query strips of 512 per batch
NKT = S // 128        # 16 key tiles of 128 per batch
NQT = 512 // 128      # 4 query tiles per strip
INV_SQRT_HD = 1.0 / np.sqrt(np.float32(HD))

F32 = dt.float32
F32R = dt.float32r

_CACHE = {}


def build_program():
    if "nc" in _CACHE:
        return _CACHE["nc"]

    nc = bacc.Bacc("TRN2", target_bir_lowering=False, debug=False,
                   num_devices=N_CORES)

    xT = nc.declare_dram_parameter("xT", [D, SG], F32R, isOutput=False)
    wqT = nc.declare_dram_parameter("wqT", [D, HD], F32R, isOutput=False)
    wkT = nc.declare_dram_parameter("wkT", [D, HD], F32R, isOutput=False)
    wvT = nc.declare_dram_parameter("wvT", [D, HD], F32R, isOutput=False)
    woT = nc.declare_dram_parameter("woT", [HD, D], F32R, isOutput=False)
    cT = nc.declare_dram_parameter("cT", [D, ROWS_PER_CORE], F32R, isOutput=False)
    ldT = nc.declare_dram_parameter("ldT", [D, RANK], F32R, isOutput=False)
    luT = nc.declare_dram_parameter("luT", [RANK, D], F32R, isOutput=False)
    bo = nc.declare_dram_parameter("bo", [1, D], F32, isOutput=False)
    out = nc.declare_dram_parameter("out", [SG, D], F32, isOutput=True)
    lora_out = nc.declare_dram_parameter("lora_out", [ROWS_PER_CORE, D], F32,
                                         isOutput=True)

    with TileContext(nc) as tc:
        with tc.tile_pool(name="const", bufs=1) as constp, \
             tc.tile_pool(name="wts", bufs=1) as wts, \
             tc.tile_pool(name="op_ps", bufs=2, space="PSUM") as op_ps, \
             tc.tile_pool(name="sc_ps", bufs=2, space="PSUM") as sc_ps, \
             tc.tile_pool(name="at_ps", bufs=1, space="PSUM") as at_ps, \
             tc.tile_pool(name="sum_ps", bufs=1, space="PSUM") as sum_ps, \
             tc.tile_pool(name="xt", bufs=2) as xtp, \
             tc.tile_pool(name="qkv", bufs=2) as qkvp, \
             tc.tile_pool(name="es", bufs=5) as esp, \
             tc.tile_pool(name="small", bufs=2) as smallp, \
             tc.tile_pool(name="outp", bufs=2) as outp, \
             tc.tile_pool(name="ct", bufs=1) as ctp:

            # ---- constants & weight loads (smallest first) ----
            ident = constp.tile([128, 128], F32, tag="ident")
            make_identity(nc, ident[:])
            ones_f = constp.tile([128, 1], F32, tag="onesf")
            nc.vector.memset(ones_f[:], 1.0)
            ones = constp.tile([128, 1], F32R, tag="ones")
            nc.vector.tensor_copy(ones[:], ones_f[:])

            wq_sb = wts.tile([128, D], F32R, tag="wq")
            wk_sb = wts.tile([128, D], F32R, tag="wk")
            wv_sb = wts.tile([128, D], F32R, tag="wv")
            wo_sb = wts.tile([HD, D], F32R, tag="wo")
            lu_sb = wts.tile([RANK, D], F32R, tag="lu")
            ld_sb = wts.tile([128, 8 * RANK], F32R, tag="ld")
            bo_sb = wts.tile([1, D], F32, tag="bo")
            bo_bc = wts.tile([128, D], F32, tag="bobc")
            for _w_sb, _wT in ((wq_sb, wqT), (wk_sb, wkT), (wv_sb, wvT)):
                nc.sync.dma_start(
                    out=_w_sb[:].rearrange("p (t m) -> p t m", t=8),
                    in_=_wT[:].rearrange("(t p) m -> p t m", p=128))

            xt_tiles = {}

            def load_xt(b):
                tiles = [xtp.tile([128, 4 * S], F32R, tag="xt",
                                  name=f"xt{b}_{h}") for h in range(2)]
                for h in range(2):
                    nc.sync.dma_start(
                        out=tiles[h][:].rearrange("p (t s) -> p t s", t=4),
                        in_=xT[h * 512:(h + 1) * 512, b * S:(b + 1) * S]
                        .rearrange("(t p) s -> p t s", p=128))
                xt_tiles[b] = tiles

            load_xt(0)

            ct_sb = ctp.tile([128, 8 * ROWS_PER_CORE], F32R, tag="ct")
            nc.sync.dma_start(
                out=ct_sb[:].rearrange("p (t m) -> p t m", t=8),
                in_=cT[:].rearrange("(t p) m -> p t m", p=128))
            nc.sync.dma_start(out=wo_sb[:], in_=woT[:])
            nc.sync.dma_start(out=lu_sb[:], in_=luT[:])
            nc.sync.dma_start(out=ld_sb[:].rearrange("p (t m) -> p t m", t=8),
                              in_=ldT[:].rearrange("(t p) m -> p t m", p=128))
            nc.sync.dma_start(out=bo_sb[:], in_=bo[:])
            nc.gpsimd.partition_broadcast(bo_bc[:], bo_sb[:])

            # PE warmup while the first activation DMAs land: keeps the HAM
            # clock-gate warm and fills the otherwise-idle load window.
            wu_ps = sum_ps.tile([1, 512], F32, tag="sums")
            for _wu in range(48):
                nc.tensor.matmul(wu_ps[:], ones[:], wq_sb[:, 0:512],
                                 start=True, stop=True)

            def emit_lora():
                # rows [c*512, (c+1)*512) of up(down(ctrl)) + bias
                dn_ps = op_ps.tile([RANK, ROWS_PER_CORE], F32, tag="op",
                                   name="dn_ps")
                for d in range(8):
                    nc.tensor.matmul(
                        dn_ps[:],
                        ld_sb[:, d * RANK:(d + 1) * RANK],
                        ct_sb[:, d * ROWS_PER_CORE:(d + 1) * ROWS_PER_CORE],
                        start=(d == 0), stop=(d == 7))
                dn_sb = smallp.tile([RANK, ROWS_PER_CORE], F32R, tag="dn")
                nc.vector.tensor_copy(dn_sb[:], dn_ps[:])
                for j in range(ROWS_PER_CORE // 128):
                    lo_sb = outp.tile([128, D], F32, tag="osb", name="lo_sb")
                    for g in range(2):
                        up_ps = op_ps.tile([128, 512], F32, tag="op",
                                           name="up_ps")
                        nc.tensor.matmul(
                            up_ps[:],
                            dn_sb[:, j * 128:(j + 1) * 128],
                            lu_sb[:, g * 512:(g + 1) * 512],
                            start=True, stop=True)
                        nc.vector.tensor_add(
                            lo_sb[:, g * 512:(g + 1) * 512], up_ps[:],
                            bo_bc[:, g * 512:(g + 1) * 512])
                    nc.sync.dma_start(out=lora_out[j * 128:(j + 1) * 128, :],
                                      in_=lo_sb[:])

            def emit_qkv(b):
                if b not in xt_tiles:
                    load_xt(b)
                xt = xt_tiles[b]

                qt_sb = qkvp.tile([HD, S], F32R, tag="qt", name=f"qt{b}")
                kt_sb = qkvp.tile([HD, S], F32R, tag="kt", name=f"kt{b}")
                vt_sb = qkvp.tile([HD, S], F32, tag="vt", bufs=1,
                                  name=f"vt{b}")
                v_sb = qkvp.tile([128, S], F32R, tag="v", name=f"v{b}")

                def proj_half(w_sb, dst, strip, h):
                    # contraction split in two PSUM groups per strip so the
                    # h=0 half can run while the second xT half still loads
                    ps2 = sc_ps.tile([128, 1024], F32, tag="sc", name="ps2")
                    ps = ps2[:, 0:512]
                    for dl in range(4):
                        d = h * 4 + dl
                        nc.tensor.matmul(
                            ps,
                            w_sb[:, d * HD:(d + 1) * HD],
                            xt[h][:, dl * S + strip * 512:
                                    dl * S + strip * 512 + 512],
                            start=(dl == 0), stop=(dl == 3),
                            skip_group_check=True)
                    sl = slice(strip * 512, (strip + 1) * 512)
                    if h == 0:
                        nc.vector.tensor_copy(dst[:, sl], ps)
                    else:
                        nc.vector.tensor_add(dst[:, sl], ps, dst[:, sl])

                for strip in range(NSTRIP):
                    proj_half(wv_sb, vt_sb, strip, 0)
                for strip in range(NSTRIP):
                    proj_half(wq_sb, qt_sb, strip, 0)
                for strip in range(NSTRIP):
                    proj_half(wk_sb, kt_sb, strip, 0)
                # second halves; V transposes interleave to keep PE dense
                for strip in range(NSTRIP):
                    proj_half(wv_sb, vt_sb, strip, 1)
                for strip in range(NSTRIP):
                    proj_half(wq_sb, qt_sb, strip, 1)
                    tq2 = sc_ps.tile([128, 1024], F32, tag="sc", name="tq2")
                    for i, kt in enumerate(range(4 * strip, 4 * strip + 4)):
                        nc.tensor.transpose(
                            tq2[:, i * 128:(i + 1) * 128],
                            vt_sb[:, kt * 128:(kt + 1) * 128], ident[:])
                    nc.vector.tensor_copy(
                        v_sb[:, strip * 512:(strip + 1) * 512],
                        tq2[:, 0:512])
                for strip in range(NSTRIP):
                    proj_half(wk_sb, kt_sb, strip, 1)
                return qt_sb, kt_sb, v_sb

            def emit_attention(b, qt_sb, kt_sb, v_sb):
                for strip in range(NSTRIP):
                    q_sl = slice(strip * 512, (strip + 1) * 512)
                    at_ps_t = at_ps.tile([HD, 512], F32, tag="at",
                                         name="at_ps_t")
                    sm_ps = sum_ps.tile([1, 512], F32, tag="sums",
                                        name="sm_ps")
                    for p in range(NKT // 2):
                        kt0 = 2 * p
                        scp = sc_ps.tile([128, 1024], F32, tag="sc",
                                         name="scp")
                        for i in range(2):
                            nc.tensor.matmul(
                                scp[:, i * 512:(i + 1) * 512],
                                kt_sb[:, (kt0 + i) * 128:(kt0 + i + 1) * 128],
                                qt_sb[:, q_sl],
                                start=True, stop=True,
                                skip_group_check=True)
                        es2 = esp.tile([128, 1024], F32R, tag="es",
                                       name="es2")
                        nc.scalar.activation(
                            es2[:], scp[:], mybir.ActivationFunctionType.Exp,
                            scale=float(INV_SQRT_HD))
                        for i in range(2):
                            kt = kt0 + i
                            nc.tensor.matmul(
                                at_ps_t[:],
                                v_sb[:, kt * 128:(kt + 1) * 128],
                                es2[:, i * 512:(i + 1) * 512],
                                start=(kt == 0), stop=(kt == NKT - 1),
                                skip_group_check=True)
                            nc.tensor.matmul(
                                sm_ps[:],
                                ones[:],
                                es2[:, i * 512:(i + 1) * 512],
                                start=(kt == 0), stop=(kt == NKT - 1),
                                skip_group_check=True)

                    # sums [1,512] -> SBUF row -> scatter to [128,4] columns
                    # -> 128-lane reciprocal (a [1,512] reciprocal would run
                    # serially on one DVE lane, ~3.3us)
                    row_sm = smallp.tile([1, 512], F32, tag="rowsm",
                                         name="row_sm")
                    nc.vector.tensor_copy(row_sm[:], sm_ps[:])
                    rcol_sb = smallp.tile([128, NQT], F32, tag="rcol",
                                          name="rcol_sb")
                    for j in range(NQT):
                        nc.sync.dma_start(
                            out=rcol_sb[:, j:j + 1],
                            in_=row_sm[0:1, j * 128:(j + 1) * 128])
                    rc_sb = smallp.tile([128, NQT], F32, tag="rc",
                                        name="rc_sb")
                    nc.vector.reciprocal(rc_sb[:], rcol_sb[:])

                    atn_sb = smallp.tile([HD, 512], F32R, tag="atn",
                                         name="atn_sb")
                    nc.vector.tensor_copy(atn_sb[:], at_ps_t[:])

                    # output projection + deferred softmax normalization
                    for j in range(NQT):
                        o_sb = outp.tile([128, D], F32, tag="osb", name="o_sb")
                        for g in range(2):
                            op = op_ps.tile([128, 512], F32, tag="op",
                                            name="op")
                            nc.tensor.matmul(
                                op[:],
                                atn_sb[:, j * 128:(j + 1) * 128],
                                wo_sb[:, g * 512:(g + 1) * 512],
                                start=True, stop=True)
                            nc.vector.tensor_scalar_mul(
                                o_sb[:, g * 512:(g + 1) * 512], op[:],
                                rc_sb[:, j:j + 1])
                        r0 = b * S + strip * 512 + j * 128
                        nc.sync.dma_start(out=out[r0:r0 + 128, :], in_=o_sb[:])

            qkv0 = emit_qkv(0)
            emit_lora()
            qkv1 = emit_qkv(1)
            emit_attention(0, *qkv0)
            emit_attention(1, *qkv1)

    nc.compile()
    _CACHE["nc"] = nc
    return nc


def _prep_in_maps(inputs):
    hidden = np.ascontiguousarray(inputs["hidden_states"], dtype=np.float32)
    control = np.ascontiguousarray(inputs["control_states"], dtype=np.float32)
    Wq = np.asarray(inputs["Wq"], dtype=np.float32)
    Wk = np.asarray(inputs["Wk"], dtype=np.float32)
    Wv = np.asarray(inputs["Wv"], dtype=np.float32)
    Wo = np.asarray(inputs["Wo"], dtype=np.float32)
    bo = np.asarray(inputs["bo"], dtype=np.float32)
    ld = np.asarray(inputs["lora_down"], dtype=np.float32)
    lu = np.asarray(inputs["lora_up"], dtype=np.float32)

    xT = np.ascontiguousarray(hidden.reshape(SG, D).T)
    cT_full = np.ascontiguousarray(control.reshape(SG, D).T)
    ldT = np.ascontiguousarray(ld.T)
    luT = np.ascontiguousarray(lu.T)
    bo_in = np.ascontiguousarray(bo.reshape(1, D))

    in_maps = []
    for c in range(N_CORES):
        hs = slice(c * HD, (c + 1) * HD)
        rs = slice(c * ROWS_PER_CORE, (c + 1) * ROWS_PER_CORE)
        in_maps.append({
            "xT": xT,
            "wqT": np.ascontiguousarray(Wq[hs, :].T),
            "wkT": np.ascontiguousarray(Wk[hs, :].T),
            "wvT": np.ascontiguousarray(Wv[hs, :].T),
            "woT": np.ascontiguousarray(Wo[:, hs].T),
            "cT": np.ascontiguousarray(cT_full[:, rs]),
            "ldT": ldT,
            "luT": luT,
            "bo": bo_in,
        })
    return in_maps


def _reduce_outputs(results):
    total = np.zeros((SG, D), dtype=np.float64)
    for c in range(N_CORES):
        total += results[c]["out"].astype(np.float64)
    total = total.astype(np.float32)
    for c in range(N_CORES):
        rs = slice(c * ROWS_PER_CORE, (c + 1) * ROWS_PER_CORE)
        total[rs] += results[c]["lora_out"]
    return total.reshape(B, S, D)


def kernel(**inputs):
    nc = build_program()
    in_maps = _prep_in_maps(inputs)
    res = run_bass_kernel_spmd(nc, in_maps, list(range(N_CORES)))
    return _reduce_outputs(res.results)



# revision 14
# speedup vs baseline: 1.0560x; 1.0560x over previous
"""Trainium2 Bass kernel for ControlLoRACrossAttnProcessor.

Head-parallel sharding over 8 NeuronCores: core c owns attention head c.
Each core computes its head's full attention and a partial output
projection; the host sums the 8 partials. The rank-4 LoRA path is
sharded by sequence rows and returned as a separate output.

Numerics: bf16 matmul operands with fp32 PSUM accumulation everywhere,
except the probs@V and probs-rowsum matmuls which run fp8e4 in DoubleRow
perf mode (0.5 cycles/row, 2x128 contraction per pass). V is quantized
as an fp8 (hi, lo) pair with both products accumulated in the same PSUM
group, so the V quantization error cancels to ~0.1%; the remaining error
is dominated by fp8 probs (~1e-2 end-to-end, gate is 2e-2).

Scheduling: per strip, the scores matmuls of pair p+1 are emitted before
the attn/sums matmuls of pair p so the PE streams while ACT computes
exp(p); the output projection of strip s-1 (and the LoRA path during the
first strip) is interleaved into the pair loop as PE filler; softmax
normalization is deferred into the output-projection PSUM->SBUF copies.
"""

import sys

for _p in ("/opt/trn_rl_repo", "/root/.axon_site"):
    if _p not in sys.path:
        sys.path.insert(0, _p)

import numpy as np
import ml_dtypes

import concourse.bass as bass  # noqa: E402
import concourse.mybir as mybir  # noqa: E402
from concourse import bacc  # noqa: E402
from concourse.bass_utils import run_bass_kernel_spmd  # noqa: E402
from concourse.tile import TileContext  # noqa: E402

dt = mybir.dt

B, S, D = 2, 2048, 1024
H = 8
HD = 128
RANK = 4
N_CORES = 8
SG = B * S
ROWS_PER_CORE = SG // N_CORES  # 512
NSTRIP = S // 512     # 4 query strips of 512 per batch
NKT = S // 128        # 16 key tiles of 128 per batch
NPAIR = NKT // 2      # 8 key-tile pairs per strip
NQT = 512 // 128      # 4 query tiles per strip
INV_SQRT_HD = 1.0 / np.sqrt(np.float32(HD))

F32 = dt.float32
BF16 = dt.bfloat16
FP8 = dt.float8e4
DR = mybir.MatmulPerfMode.DoubleRow

_CACHE = {}


def build_program(debug=False):
    key = f"nc{debug}"
    if key in _CACHE:
        return _CACHE[key]

    nc = bacc.Bacc("TRN2", target_bir_lowering=False, debug=False,
                   num_devices=N_CORES)

    xT = nc.declare_dram_parameter("xT", [D, SG], BF16, isOutput=False)
    wqT = nc.declare_dram_parameter("wqT", [D, HD], BF16, isOutput=False)
    wkT = nc.declare_dram_parameter("wkT", [D, HD], BF16, isOutput=False)
    wvT = nc.declare_dram_parameter("wvT", [D, HD], BF16, isOutput=False)
    woT = nc.declare_dram_parameter("woT", [HD, D], BF16, isOutput=False)
    cT = nc.declare_dram_parameter("cT", [D, ROWS_PER_CORE], BF16,
                                   isOutput=False)
    ldT = nc.declare_dram_parameter("ldT", [D, RANK], BF16, isOutput=False)
    luT = nc.declare_dram_parameter("luT", [RANK, D], BF16, isOutput=False)
    bo = nc.declare_dram_parameter("bo", [1, D], F32, isOutput=False)
    out = nc.declare_dram_parameter("out", [SG, D], BF16, isOutput=True)
    lora_out = nc.declare_dram_parameter("lora_out", [ROWS_PER_CORE, D],
                                         BF16, isOutput=True)
    if debug:
        dbg_q = nc.declare_dram_parameter("dbg_q", [HD, S], BF16,
                                          isOutput=True)
        dbg_k = nc.declare_dram_parameter("dbg_k", [HD, S], BF16,
                                          isOutput=True)
        dbg_vhi = nc.declare_dram_parameter("dbg_vhi", [128, NKT * HD], FP8,
                                            isOutput=True)
        dbg_vlo = nc.declare_dram_parameter("dbg_vlo", [128, NKT * HD], FP8,
                                            isOutput=True)
        dbg_es = nc.declare_dram_parameter("dbg_es", [128, NPAIR * 1024],
                                           FP8, isOutput=True)
        dbg_sums = nc.declare_dram_parameter("dbg_sums", [1, 512], F32,
                                             isOutput=True)
        dbg_rc = nc.declare_dram_parameter("dbg_rc", [128, NQT], F32,
                                           isOutput=True)
        dbg_atn = nc.declare_dram_parameter("dbg_atn", [HD, 512], BF16,
                                            isOutput=True)

    with TileContext(nc) as tc:
        with tc.tile_pool(name="const", bufs=1) as constp, \
             tc.tile_pool(name="wts", bufs=1) as wts, \
             tc.tile_pool(name="sc_ps", bufs=2, space="PSUM") as sc_ps, \
             tc.tile_pool(name="at_ps", bufs=1, space="PSUM") as at_ps, \
             tc.tile_pool(name="sum_ps", bufs=1, space="PSUM") as sum_ps, \
             tc.tile_pool(name="op_ps", bufs=2, space="PSUM") as op_ps, \
             tc.tile_pool(name="xt", bufs=2) as xtp, \
             tc.tile_pool(name="qkv", bufs=2) as qkvp, \
             tc.tile_pool(name="es", bufs=4) as esp, \
             tc.tile_pool(name="small", bufs=2) as smallp, \
             tc.tile_pool(name="outp", bufs=2) as outp, \
             tc.tile_pool(name="ct", bufs=1) as ctp:

            # ---- constants & weights ----
            from concourse.masks import make_identity
            ident_f = constp.tile([128, 128], F32, tag="identf")
            make_identity(nc, ident_f[:])
            ident = constp.tile([128, 128], BF16, tag="ident")
            nc.vector.tensor_copy(ident[:], ident_f[:])
            # full 128-column ones: dual-fp8 LDWEIGHTS (walrus
            # s3_lw_dual_fp8_restrictions) rejects narrow stationaries, so
            # the row-sums matmul replicates the sums across all 128 psum
            # partitions (same cycle cost — cost scales with moving rows)
            ones_dr = constp.tile([128, 2, 128], FP8, tag="ones")
            nc.vector.memset(ones_dr[:], 1.0)

            wq_sb = wts.tile([128, 8, HD], BF16, tag="wq")
            wk_sb = wts.tile([128, 8, HD], BF16, tag="wk")
            wv_sb = wts.tile([128, 8, HD], BF16, tag="wv")
            wo_sb = wts.tile([HD, D], BF16, tag="wo")
            lu_sb = wts.tile([RANK, D], BF16, tag="lu")
            ld_sb = wts.tile([128, 8, RANK], BF16, tag="ld")
            bo_sb = wts.tile([1, D], F32, tag="bo")
            bo_bc = wts.tile([128, D], F32, tag="bobc")
            for _w_sb, _wT in ((wq_sb, wqT), (wk_sb, wkT), (wv_sb, wvT)):
                nc.sync.dma_start(
                    out=_w_sb[:],
                    in_=_wT[:].rearrange("(t p) m -> p t m", p=128))
            nc.sync.dma_start(out=wo_sb[:], in_=woT[:])
            nc.sync.dma_start(out=lu_sb[:], in_=luT[:])
            nc.sync.dma_start(out=ld_sb[:],
                              in_=ldT[:].rearrange("(t p) m -> p t m", p=128))
            nc.sync.dma_start(out=bo_sb[:], in_=bo[:])
            nc.gpsimd.partition_broadcast(bo_bc[:], bo_sb[:])

            # ---- activation loads (x in per-strip chunks for fast start) --
            xt_tiles = {}

            def load_xt(b):
                tiles = [xtp.tile([128, 4, S], BF16, tag="xt",
                                  name=f"xt{b}_{h}") for h in range(2)]
                for h in range(2):
                    for sch in range(NSTRIP):
                        nc.sync.dma_start(
                            out=tiles[h][:, :, sch * 512:(sch + 1) * 512],
                            in_=xT[h * 512:(h + 1) * 512,
                                   b * S + sch * 512:b * S + sch * 512 + 512]
                            .rearrange("(t p) s -> p t s", p=128))
                xt_tiles[b] = tiles

            load_xt(0)

            ct_sb = ctp.tile([128, 8, ROWS_PER_CORE], BF16, tag="ct")
            nc.sync.dma_start(
                out=ct_sb[:],
                in_=cT[:].rearrange("(t p) m -> p t m", p=128))

            # PE warmup while first DMAs land (also ramps the p-state)
            wu_ps = sum_ps.tile([1, 512], F32, tag="sums")
            for _wu in range(32):
                nc.tensor.matmul(wu_ps[:], ld_sb[:, 0, 0:1],
                                 ct_sb[:, 0, 0:512], start=True, stop=True)

            # ---------- LoRA path (emitted as PE filler thunks) ----------
            def lora_thunks():
                thunks = []
                dn_ps = op_ps.tile([RANK, ROWS_PER_CORE], F32, tag="op",
                                   name="dn_ps")
                for d in range(8):
                    def t(d=d):
                        nc.tensor.matmul(
                            dn_ps[:], ld_sb[:, d, :], ct_sb[:, d, :],
                            start=(d == 0), stop=(d == 7))
                    thunks.append(t)
                dn_sb = smallp.tile([RANK, ROWS_PER_CORE], BF16, tag="dn")

                def t_cp():
                    nc.vector.tensor_copy(dn_sb[:], dn_ps[:])
                thunks.append(t_cp)
                for j in range(ROWS_PER_CORE // 128):
                    lo_sb = outp.tile([128, D], BF16, tag="osb",
                                      name="lo_sb")
                    for g in range(2):
                        def t(j=j, g=g, lo_sb=lo_sb):
                            up_ps = op_ps.tile([128, 512], F32, tag="op",
                                               name="up_ps")
                            nc.tensor.matmul(
                                up_ps[:], dn_sb[:, j * 128:(j + 1) * 128],
                                lu_sb[:, g * 512:(g + 1) * 512],
                                start=True, stop=True)
                            nc.vector.tensor_add(
                                lo_sb[:, g * 512:(g + 1) * 512], up_ps[:],
                                bo_bc[:, g * 512:(g + 1) * 512])
                        thunks.append(t)

                    def t_st(j=j, lo_sb=lo_sb):
                        nc.sync.dma_start(
                            out=lora_out[j * 128:(j + 1) * 128, :],
                            in_=lo_sb[:])
                    thunks.append(t_st)
                return thunks

            # ---------- QKV projections ----------
            def emit_qk(b, halves):
                """q/k projections for batch b -> qt/kt [128(d), S] bf16.

                halves=True: two PSUM groups per strip (copy then add) so
                compute starts before the second half of x lands.
                """
                if b not in xt_tiles:
                    load_xt(b)
                xt = xt_tiles[b]
                qt_sb = qkvp.tile([HD, S], BF16, tag="qt", name=f"qt{b}")
                kt_sb = qkvp.tile([HD, S], BF16, tag="kt", name=f"kt{b}")

                def proj(w_sb, dst, strip, h0, h1, first):
                    ps = sc_ps.tile([128, 1024], F32, tag="sc", name="qkps")
                    for h in range(h0, h1):
                        for dl in range(4):
                            nc.tensor.matmul(
                                ps[:, 0:512],
                                w_sb[:, h * 4 + dl, :],
                                xt[h][:, dl, strip * 512:strip * 512 + 512],
                                start=(h == h0 and dl == 0),
                                stop=(h == h1 - 1 and dl == 3),
                                skip_group_check=True)
                    sl = slice(strip * 512, (strip + 1) * 512)
                    if first and h1 == 2:
                        nc.vector.tensor_copy(dst[:, sl], ps[:, 0:512])
                        return
                    if first:
                        nc.vector.tensor_copy(dst[:, sl], ps[:, 0:512])
                    else:
                        nc.vector.tensor_add(dst[:, sl], ps[:, 0:512],
                                             dst[:, sl])

                if halves:
                    for strip in range(NSTRIP):
                        proj(wq_sb, qt_sb, strip, 0, 1, True)
                    for strip in range(NSTRIP):
                        proj(wk_sb, kt_sb, strip, 0, 1, True)
                    for strip in range(NSTRIP):
                        proj(wq_sb, qt_sb, strip, 1, 2, False)
                    for strip in range(NSTRIP):
                        proj(wk_sb, kt_sb, strip, 1, 2, False)
                else:
                    for strip in range(NSTRIP):
                        proj(wq_sb, qt_sb, strip, 0, 2, True)
                    for strip in range(NSTRIP):
                        proj(wk_sb, kt_sb, strip, 0, 2, True)
                return qt_sb, kt_sb

            def emit_v(b):
                """V for batch b as fp8 (hi, lo) pair in [k, d] layout.

                Projects strip-wise into vt [d, S] bf16, PE-transposes each
                [128,128] tile, then quantizes hi = fp8(v),
                lo = fp8(v - hi); hi+lo accumulate in one PSUM group later.
                """
                xt = xt_tiles[b]
                vt_sb = qkvp.tile([HD, S], BF16, tag="vt", bufs=1,
                                  name=f"vt{b}")
                v_hi = qkvp.tile([128, NKT, HD], FP8, tag="vhi",
                                 name=f"vhi{b}")
                v_lo = qkvp.tile([128, NKT, HD], FP8, tag="vlo",
                                 name=f"vlo{b}")
                for strip in range(NSTRIP):
                    ps = sc_ps.tile([128, 1024], F32, tag="sc", name="vps")
                    for h in range(2):
                        for dl in range(4):
                            nc.tensor.matmul(
                                ps[:, 0:512],
                                wv_sb[:, h * 4 + dl, :],
                                xt[h][:, dl, strip * 512:strip * 512 + 512],
                                start=(h == 0 and dl == 0),
                                stop=(h == 1 and dl == 3),
                                skip_group_check=True)
                    sl = slice(strip * 512, (strip + 1) * 512)
                    nc.vector.tensor_copy(vt_sb[:, sl], ps[:, 0:512])
                for strip in range(NSTRIP):
                    tq = at_ps.tile([128, 512], BF16, tag="at", name="tq")
                    for i in range(4):
                        kt = 4 * strip + i
                        nc.tensor.transpose(
                            tq[:, i * 128:(i + 1) * 128],
                            vt_sb[:, kt * 128:(kt + 1) * 128], ident[:])
                    hi = v_hi[:, 4 * strip:4 * strip + 4, :]
                    lo = v_lo[:, 4 * strip:4 * strip + 4, :]
                    hi2 = hi.rearrange("p t m -> p (t m)")
                    lo2 = lo.rearrange("p t m -> p (t m)")
                    nc.vector.tensor_copy(hi2, tq[:])
                    nc.vector.tensor_tensor(
                        lo2, tq[:], hi2, mybir.AluOpType.subtract)
                return v_hi, v_lo

            # ---------- attention ----------
            def emit_attention(b, qt_sb, kt_sb, v_hi, v_lo, filler):
                """filler: list of thunks to interleave as PE bubble filler.

                Per strip: scores(p+1) emitted before attn/sums(p); the
                output projection of strip s-1 is appended to the filler
                queue and drained one thunk per pair.
                """
                pending = list(filler)

                def drain(n):
                    for _ in range(n):
                        if pending:
                            pending.pop(0)()

                def outproj_thunks(strip, atn_sb, rc_sb):
                    thunks = []
                    for j in range(NQT):
                        o_sb = outp.tile([128, D], BF16, tag="osb",
                                         name="o_sb")
                        for g in range(2):
                            def t(j=j, g=g, o_sb=o_sb):
                                op = op_ps.tile([128, 512], F32, tag="op",
                                                name="op")
                                nc.tensor.matmul(
                                    op[:],
                                    atn_sb[:, j * 128:(j + 1) * 128],
                                    wo_sb[:, g * 512:(g + 1) * 512],
                                    start=True, stop=True)
                                nc.vector.tensor_scalar_mul(
                                    o_sb[:, g * 512:(g + 1) * 512], op[:],
                                    rc_sb[:, j:j + 1])
                            thunks.append(t)

                        def t_st(j=j, o_sb=o_sb, strip=strip):
                            r0 = b * S + strip * 512 + j * 128
                            nc.sync.dma_start(out=out[r0:r0 + 128, :],
                                              in_=o_sb[:])
                        thunks.append(t_st)
                    return thunks

                for strip in range(NSTRIP):
                    q_sl = slice(strip * 512, (strip + 1) * 512)
                    at_t = at_ps.tile([128, 512], F32, tag="at",
                                      name="at_t")
                    sm_ps = sum_ps.tile([128, 512], F32, tag="sums",
                                        name="sm_ps")
                    es_tiles = []

                    def scores(p):
                        scp = sc_ps.tile([128, 1024], F32, tag="sc",
                                         name="scp")
                        for i in range(2):
                            kt = 2 * p + i
                            nc.tensor.matmul(
                                scp[:, i * 512:(i + 1) * 512],
                                kt_sb[:, kt * 128:(kt + 1) * 128],
                                qt_sb[:, q_sl],
                                start=True, stop=True,
                                skip_group_check=True)
                        es2 = esp.tile([128, 2, 512], FP8, tag="es",
                                       name="es2")
                        nc.scalar.activation(
                            es2[:].rearrange("p t n -> p (t n)"), scp[:],
                            mybir.ActivationFunctionType.Exp,
                            scale=float(INV_SQRT_HD))
                        if debug and b == 0 and strip == 0:
                            nc.sync.dma_start(
                                out=dbg_es[:, p * 1024:(p + 1) * 1024],
                                in_=es2[:].rearrange("p t n -> p (t n)"))
                        es_tiles.append(es2)

                    def attnsum(p):
                        es2 = es_tiles[p]
                        nc.tensor.matmul(
                            at_t[:], v_hi[:, 2 * p:2 * p + 2, :], es2[:],
                            start=(p == 0), stop=False,
                            perf_mode=DR, skip_group_check=True)
                        nc.tensor.matmul(
                            at_t[:], v_lo[:, 2 * p:2 * p + 2, :], es2[:],
                            start=False, stop=(p == NPAIR - 1),
                            perf_mode=DR, skip_group_check=True)
                        nc.tensor.matmul(
                            sm_ps[:], ones_dr[:], es2[:],
                            start=(p == 0), stop=(p == NPAIR - 1),
                            perf_mode=DR, skip_group_check=True)

                    for p in range(NPAIR):
                        scores(p)
                        drain(1)
                        if p >= 1:
                            attnsum(p - 1)
                    attnsum(NPAIR - 1)

                    # softmax denominators: [1,512] -> [128,4] columns ->
                    # 128-lane reciprocal; scaling rides the outproj copies
                    row_sm = smallp.tile([1, 512], F32, tag="rowsm",
                                         name="row_sm")
                    nc.vector.tensor_copy(row_sm[:], sm_ps[0:1, :])
                    rcol_sb = smallp.tile([128, NQT], F32, tag="rcol",
                                          name="rcol_sb")
                    for j in range(NQT):
                        nc.sync.dma_start(
                            out=rcol_sb[:, j:j + 1],
                            in_=row_sm[0:1, j * 128:(j + 1) * 128])
                    rc_sb = smallp.tile([128, NQT], F32, tag="rc",
                                        name="rc_sb")
                    nc.vector.reciprocal(rc_sb[:], rcol_sb[:])

                    atn_sb = smallp.tile([HD, 512], BF16, tag="atn",
                                         name="atn_sb")
                    nc.vector.tensor_copy(atn_sb[:], at_t[:])
                    if debug and b == 0 and strip == 0:
                        nc.sync.dma_start(out=dbg_sums[:], in_=row_sm[:])
                        nc.sync.dma_start(out=dbg_rc[:], in_=rc_sb[:])
                        nc.sync.dma_start(out=dbg_atn[:], in_=atn_sb[:])

                    drain(len(pending))
                    pending = outproj_thunks(strip, atn_sb, rc_sb)
                for t in pending:
                    t()

            # ---------- program ----------
            qk0 = emit_qk(0, halves=True)
            v0 = emit_v(0)
            if debug:
                nc.sync.dma_start(out=dbg_q[:], in_=qk0[0][:])
                nc.sync.dma_start(out=dbg_k[:], in_=qk0[1][:])
                nc.sync.dma_start(
                    out=dbg_vhi[:], in_=v0[0][:].rearrange("p t m -> p (t m)"))
                nc.sync.dma_start(
                    out=dbg_vlo[:], in_=v0[1][:].rearrange("p t m -> p (t m)"))
            emit_attention(0, *qk0, *v0, filler=lora_thunks())
            qk1 = emit_qk(1, halves=False)
            v1 = emit_v(1)
            emit_attention(1, *qk1, *v1, filler=[])

    nc.compile()
    _CACHE[key] = nc
    return nc


def _prep_in_maps(inputs):
    bf = ml_dtypes.bfloat16
    hidden = np.asarray(inputs["hidden_states"], dtype=np.float32)
    control = np.asarray(inputs["control_states"], dtype=np.float32)
    Wq = np.asarray(inputs["Wq"], dtype=np.float32)
    Wk = np.asarray(inputs["Wk"], dtype=np.float32)
    Wv = np.asarray(inputs["Wv"], dtype=np.float32)
    Wo = np.asarray(inputs["Wo"], dtype=np.float32)
    bo = np.asarray(inputs["bo"], dtype=np.float32)
    ld = np.asarray(inputs["lora_down"], dtype=np.float32)
    lu = np.asarray(inputs["lora_up"], dtype=np.float32)

    xT = np.ascontiguousarray(hidden.reshape(SG, D).T).astype(bf)
    cT_full = np.ascontiguousarray(control.reshape(SG, D).T).astype(bf)
    ldT = np.ascontiguousarray(ld.T).astype(bf)
    luT = np.ascontiguousarray(lu.T).astype(bf)
    bo_in = np.ascontiguousarray(bo.reshape(1, D))

    in_maps = []
    for c in range(N_CORES):
        hs = slice(c * HD, (c + 1) * HD)
        rs = slice(c * ROWS_PER_CORE, (c + 1) * ROWS_PER_CORE)
        in_maps.append({
            "xT": xT,
            "wqT": np.ascontiguousarray(Wq[hs, :].T).astype(bf),
            "wkT": np.ascontiguousarray(Wk[hs, :].T).astype(bf),
            "wvT": np.ascontiguousarray(Wv[hs, :].T).astype(bf),
            "woT": np.ascontiguousarray(Wo[:, hs].T).astype(bf),
            "cT": np.ascontiguousarray(cT_full[:, rs]),
            "ldT": ldT,
            "luT": luT,
            "bo": bo_in,
        })
    return in_maps


def _reduce_outputs(results):
    total = np.zeros((SG, D), dtype=np.float32)
    for c in range(N_CORES):
        total += results[c]["out"].astype(np.float32)
    for c in range(N_CORES):
        rs = slice(c * ROWS_PER_CORE, (c + 1) * ROWS_PER_CORE)
        total[rs] += results[c]["lora_out"].astype(np.float32)
    return total.reshape(B, S, D)


def kernel(**inputs):
    nc = build_program()
    in_maps = _prep_in_maps(inputs)
    res = run_bass_kernel_spmd(nc, in_maps, list(range(N_CORES)))
    return _reduce_outputs(res.results)


# revision 16
# speedup vs baseline: 1.0599x; 1.0037x over previous
"""Trainium2 Bass kernel for ControlLoRACrossAttnProcessor.

Head-parallel sharding over 8 NeuronCores: core c owns attention head c.
Each core computes its head's full attention and a partial output
projection; the host sums the 8 partials. The rank-4 LoRA path is
sharded by sequence rows and returned as a separate output.

Numerics: bf16 matmul operands with fp32 PSUM accumulation everywhere,
except the probs@V and probs-rowsum matmuls which run fp8e4 in DoubleRow
perf mode (0.5 cycles/row, 2x128 contraction per pass). V is quantized
as an fp8 (hi, lo) pair with both products accumulated in the same PSUM
group, so the V quantization error cancels to ~0.1%; the remaining error
is dominated by fp8 probs (~1e-2 end-to-end, gate is 2e-2).

Scheduling: per strip, the scores matmuls of pair p+1 are emitted before
the attn/sums matmuls of pair p so the PE streams while ACT computes
exp(p); the output projection of strip s-1 (and the LoRA path during the
first strip) is interleaved into the pair loop as PE filler; softmax
normalization is deferred into the output-projection PSUM->SBUF copies.
"""

import sys

for _p in ("/opt/trn_rl_repo", "/root/.axon_site"):
    if _p not in sys.path:
        sys.path.insert(0, _p)

import numpy as np
import ml_dtypes

import concourse.bass as bass  # noqa: E402
import concourse.mybir as mybir  # noqa: E402
from concourse import bacc  # noqa: E402
from concourse.bass_utils import run_bass_kernel_spmd  # noqa: E402
from concourse.tile import TileContext  # noqa: E402

dt = mybir.dt

B, S, D = 2, 2048, 1024
H = 8
HD = 128
RANK = 4
N_CORES = 8
SG = B * S
ROWS_PER_CORE = SG // N_CORES  # 512
NSTRIP = S // 512     # 4 query strips of 512 per batch
NKT = S // 128        # 16 key tiles of 128 per batch
NPAIR = NKT // 2      # 8 key-tile pairs per strip
NQT = 512 // 128      # 4 query tiles per strip
INV_SQRT_HD = 1.0 / np.sqrt(np.float32(HD))

F32 = dt.float32
BF16 = dt.bfloat16
FP8 = dt.float8e4
DR = mybir.MatmulPerfMode.DoubleRow

_CACHE = {}


def build_program(debug=False):
    key = f"nc{debug}"
    if key in _CACHE:
        return _CACHE[key]

    nc = bacc.Bacc("TRN2", target_bir_lowering=False, debug=False,
                   num_devices=N_CORES)

    xT = nc.declare_dram_parameter("xT", [D, SG], BF16, isOutput=False)
    wqT = nc.declare_dram_parameter("wqT", [D, HD], BF16, isOutput=False)
    wkT = nc.declare_dram_parameter("wkT", [D, HD], BF16, isOutput=False)
    wvT = nc.declare_dram_parameter("wvT", [D, HD], BF16, isOutput=False)
    woT = nc.declare_dram_parameter("woT", [HD, D], BF16, isOutput=False)
    cT = nc.declare_dram_parameter("cT", [D, ROWS_PER_CORE], BF16,
                                   isOutput=False)
    ldT = nc.declare_dram_parameter("ldT", [D, RANK], BF16, isOutput=False)
    luT = nc.declare_dram_parameter("luT", [RANK, D], BF16, isOutput=False)
    bo = nc.declare_dram_parameter("bo", [1, D], F32, isOutput=False)
    out = nc.declare_dram_parameter("out", [SG, D], BF16, isOutput=True)
    lora_out = nc.declare_dram_parameter("lora_out", [ROWS_PER_CORE, D],
                                         BF16, isOutput=True)
    if debug:
        dbg_q = nc.declare_dram_parameter("dbg_q", [HD, S], BF16,
                                          isOutput=True)
        dbg_k = nc.declare_dram_parameter("dbg_k", [HD, S], BF16,
                                          isOutput=True)
        dbg_vhi = nc.declare_dram_parameter("dbg_vhi", [128, NKT * HD], FP8,
                                            isOutput=True)
        dbg_vlo = nc.declare_dram_parameter("dbg_vlo", [128, NKT * HD], FP8,
                                            isOutput=True)
        dbg_es = nc.declare_dram_parameter("dbg_es", [128, NPAIR * 1024],
                                           FP8, isOutput=True)
        dbg_sums = nc.declare_dram_parameter("dbg_sums", [1, 512], F32,
                                             isOutput=True)
        dbg_rc = nc.declare_dram_parameter("dbg_rc", [128, NQT], F32,
                                           isOutput=True)
        dbg_atn = nc.declare_dram_parameter("dbg_atn", [HD, 512], BF16,
                                            isOutput=True)

    with TileContext(nc) as tc:
        with tc.tile_pool(name="const", bufs=1) as constp, \
             tc.tile_pool(name="wts", bufs=1) as wts, \
             tc.tile_pool(name="sc_ps", bufs=2, space="PSUM") as sc_ps, \
             tc.tile_pool(name="at_ps", bufs=1, space="PSUM") as at_ps, \
             tc.tile_pool(name="sum_ps", bufs=1, space="PSUM") as sum_ps, \
             tc.tile_pool(name="op_ps", bufs=2, space="PSUM") as op_ps, \
             tc.tile_pool(name="xt", bufs=2) as xtp, \
             tc.tile_pool(name="qkv", bufs=2) as qkvp, \
             tc.tile_pool(name="es", bufs=4) as esp, \
             tc.tile_pool(name="small", bufs=2) as smallp, \
             tc.tile_pool(name="outp", bufs=2) as outp, \
             tc.tile_pool(name="ct", bufs=1) as ctp:

            # ---- constants & weights ----
            from concourse.masks import make_identity
            ident_f = constp.tile([128, 128], F32, tag="identf")
            make_identity(nc, ident_f[:])
            ident = constp.tile([128, 128], BF16, tag="ident")
            nc.vector.tensor_copy(ident[:], ident_f[:])
            # full 128-column ones: dual-fp8 LDWEIGHTS (walrus
            # s3_lw_dual_fp8_restrictions) rejects narrow stationaries, so
            # the row-sums matmul replicates the sums across all 128 psum
            # partitions (same cycle cost — cost scales with moving rows)
            ones_dr = constp.tile([128, 2, 128], FP8, tag="ones")
            nc.vector.memset(ones_dr[:], 1.0)

            wq_sb = wts.tile([128, 8, HD], BF16, tag="wq")
            wk_sb = wts.tile([128, 8, HD], BF16, tag="wk")
            wv_sb = wts.tile([128, 8, HD], BF16, tag="wv")
            wo_sb = wts.tile([HD, D], BF16, tag="wo")
            lu_sb = wts.tile([RANK, D], BF16, tag="lu")
            ld_sb = wts.tile([128, 8, RANK], BF16, tag="ld")
            bo_sb = wts.tile([1, D], F32, tag="bo")
            bo_bc = wts.tile([128, D], F32, tag="bobc")

            # x chunks issue on SP first (the critical path); weights go
            # through the idle Pool sequencer so both issue in parallel
            xt_tiles = {}

            def load_xt(b):
                tiles = [xtp.tile([128, 4, S], BF16, tag="xt",
                                  name=f"xt{b}_{h}") for h in range(2)]
                for h in range(2):
                    for sch in range(NSTRIP):
                        nc.sync.dma_start(
                            out=tiles[h][:, :, sch * 512:(sch + 1) * 512],
                            in_=xT[h * 512:(h + 1) * 512,
                                   b * S + sch * 512:b * S + sch * 512 + 512]
                            .rearrange("(t p) s -> p t s", p=128))
                xt_tiles[b] = tiles

            load_xt(0)

            nc.gpsimd.dma_start(out=ld_sb[:],
                                in_=ldT[:].rearrange("(t p) m -> p t m",
                                                     p=128))
            for _w_sb, _wT in ((wq_sb, wqT), (wk_sb, wkT), (wv_sb, wvT)):
                nc.gpsimd.dma_start(
                    out=_w_sb[:],
                    in_=_wT[:].rearrange("(t p) m -> p t m", p=128))
            nc.gpsimd.dma_start(out=wo_sb[:], in_=woT[:])
            nc.gpsimd.dma_start(out=lu_sb[:], in_=luT[:])
            nc.gpsimd.dma_start(out=bo_sb[:], in_=bo[:])
            nc.gpsimd.partition_broadcast(bo_bc[:], bo_sb[:])

            ct_sb = ctp.tile([128, 8, ROWS_PER_CORE], BF16, tag="ct")
            nc.sync.dma_start(
                out=ct_sb[:],
                in_=cT[:].rearrange("(t p) m -> p t m", p=128))

            # PE warmup while first DMAs land (also ramps the p-state)
            wu_ps = sum_ps.tile([1, 512], F32, tag="sums")
            for _wu in range(32):
                nc.tensor.matmul(wu_ps[:], ld_sb[:, 0, 0:1],
                                 ct_sb[:, 0, 0:512], start=True, stop=True)

            # ---------- LoRA path (emitted as PE filler thunks) ----------
            def lora_thunks():
                thunks = []
                dn_ps = op_ps.tile([RANK, ROWS_PER_CORE], F32, tag="op",
                                   name="dn_ps")
                for d in range(8):
                    def t(d=d):
                        nc.tensor.matmul(
                            dn_ps[:], ld_sb[:, d, :], ct_sb[:, d, :],
                            start=(d == 0), stop=(d == 7))
                    thunks.append(t)
                dn_sb = smallp.tile([RANK, ROWS_PER_CORE], BF16, tag="dn")

                def t_cp():
                    nc.vector.tensor_copy(dn_sb[:], dn_ps[:])
                thunks.append(t_cp)
                for j in range(ROWS_PER_CORE // 128):
                    lo_sb = outp.tile([128, D], BF16, tag="osb",
                                      name="lo_sb")
                    for g in range(2):
                        def t(j=j, g=g, lo_sb=lo_sb):
                            up_ps = op_ps.tile([128, 512], F32, tag="op",
                                               name="up_ps")
                            nc.tensor.matmul(
                                up_ps[:], dn_sb[:, j * 128:(j + 1) * 128],
                                lu_sb[:, g * 512:(g + 1) * 512],
                                start=True, stop=True)
                            nc.vector.tensor_add(
                                lo_sb[:, g * 512:(g + 1) * 512], up_ps[:],
                                bo_bc[:, g * 512:(g + 1) * 512])
                        thunks.append(t)

                    def t_st(j=j, lo_sb=lo_sb):
                        nc.sync.dma_start(
                            out=lora_out[j * 128:(j + 1) * 128, :],
                            in_=lo_sb[:])
                    thunks.append(t_st)
                return thunks

            # ---------- QKV projections ----------
            def emit_qk(b, halves):
                """q/k projections for batch b -> qt/kt [128(d), S] bf16.

                halves=True: two PSUM groups per strip (copy then add) so
                compute starts before the second half of x lands.
                """
                if b not in xt_tiles:
                    load_xt(b)
                xt = xt_tiles[b]
                qt_sb = qkvp.tile([HD, S], BF16, tag="qt", name=f"qt{b}")
                kt_sb = qkvp.tile([HD, S], BF16, tag="kt", name=f"kt{b}")

                def proj(w_sb, dst, strip, h0, h1, first):
                    ps = sc_ps.tile([128, 1024], F32, tag="sc", name="qkps")
                    for h in range(h0, h1):
                        for dl in range(4):
                            nc.tensor.matmul(
                                ps[:, 0:512],
                                w_sb[:, h * 4 + dl, :],
                                xt[h][:, dl, strip * 512:strip * 512 + 512],
                                start=(h == h0 and dl == 0),
                                stop=(h == h1 - 1 and dl == 3),
                                skip_group_check=True)
                    sl = slice(strip * 512, (strip + 1) * 512)
                    if first and h1 == 2:
                        nc.vector.tensor_copy(dst[:, sl], ps[:, 0:512])
                        return
                    if first:
                        nc.vector.tensor_copy(dst[:, sl], ps[:, 0:512])
                    else:
                        nc.vector.tensor_add(dst[:, sl], ps[:, 0:512],
                                             dst[:, sl])

                if halves:
                    for strip in range(NSTRIP):
                        proj(wq_sb, qt_sb, strip, 0, 1, True)
                    for strip in range(NSTRIP):
                        proj(wk_sb, kt_sb, strip, 0, 1, True)
                    for strip in range(NSTRIP):
                        proj(wq_sb, qt_sb, strip, 1, 2, False)
                    for strip in range(NSTRIP):
                        proj(wk_sb, kt_sb, strip, 1, 2, False)
                else:
                    for strip in range(NSTRIP):
                        proj(wq_sb, qt_sb, strip, 0, 2, True)
                    for strip in range(NSTRIP):
                        proj(wk_sb, kt_sb, strip, 0, 2, True)
                return qt_sb, kt_sb

            def emit_v(b):
                """V for batch b as fp8 (hi, lo) pair in [k, d] layout.

                Projects strip-wise into vt [d, S] bf16, PE-transposes each
                [128,128] tile, then quantizes hi = fp8(v),
                lo = fp8(v - hi); hi+lo accumulate in one PSUM group later.
                """
                xt = xt_tiles[b]
                vt_sb = qkvp.tile([HD, S], BF16, tag="vt", bufs=1,
                                  name=f"vt{b}")
                v_hi = qkvp.tile([128, NKT, HD], FP8, tag="vhi",
                                 name=f"vhi{b}")
                v_lo = qkvp.tile([128, NKT, HD], FP8, tag="vlo",
                                 name=f"vlo{b}")
                for strip in range(NSTRIP):
                    ps = sc_ps.tile([128, 1024], F32, tag="sc", name="vps")
                    for h in range(2):
                        for dl in range(4):
                            nc.tensor.matmul(
                                ps[:, 0:512],
                                wv_sb[:, h * 4 + dl, :],
                                xt[h][:, dl, strip * 512:strip * 512 + 512],
                                start=(h == 0 and dl == 0),
                                stop=(h == 1 and dl == 3),
                                skip_group_check=True)
                    sl = slice(strip * 512, (strip + 1) * 512)
                    nc.vector.tensor_copy(vt_sb[:, sl], ps[:, 0:512])
                for strip in range(NSTRIP):
                    tq = at_ps.tile([128, 512], BF16, tag="at", name="tq")
                    for i in range(4):
                        kt = 4 * strip + i
                        nc.tensor.transpose(
                            tq[:, i * 128:(i + 1) * 128],
                            vt_sb[:, kt * 128:(kt + 1) * 128], ident[:])
                    hi = v_hi[:, 4 * strip:4 * strip + 4, :]
                    lo = v_lo[:, 4 * strip:4 * strip + 4, :]
                    hi2 = hi.rearrange("p t m -> p (t m)")
                    lo2 = lo.rearrange("p t m -> p (t m)")
                    nc.vector.tensor_copy(hi2, tq[:])
                    nc.vector.tensor_tensor(
                        lo2, tq[:], hi2, mybir.AluOpType.subtract)
                return v_hi, v_lo

            # ---------- attention ----------
            def emit_attention(b, qt_sb, kt_sb, v_hi, v_lo, filler):
                """filler: list of thunks to interleave as PE bubble filler.

                Per strip: scores(p+1) emitted before attn/sums(p); the
                output projection of strip s-1 is appended to the filler
                queue and drained one thunk per pair.
                """
                pending = list(filler)

                def drain(n):
                    for _ in range(n):
                        if pending:
                            pending.pop(0)()

                def outproj_thunks(strip, atn_sb, rc_sb):
                    thunks = []
                    for j in range(NQT):
                        o_sb = outp.tile([128, D], BF16, tag="osb",
                                         name="o_sb")
                        for g in range(2):
                            def t(j=j, g=g, o_sb=o_sb):
                                op = op_ps.tile([128, 512], F32, tag="op",
                                                name="op")
                                nc.tensor.matmul(
                                    op[:],
                                    atn_sb[:, j * 128:(j + 1) * 128],
                                    wo_sb[:, g * 512:(g + 1) * 512],
                                    start=True, stop=True)
                                nc.vector.tensor_scalar_mul(
                                    o_sb[:, g * 512:(g + 1) * 512], op[:],
                                    rc_sb[:, j:j + 1])
                            thunks.append(t)

                        def t_st(j=j, o_sb=o_sb, strip=strip):
                            r0 = b * S + strip * 512 + j * 128
                            nc.sync.dma_start(out=out[r0:r0 + 128, :],
                                              in_=o_sb[:])
                        thunks.append(t_st)
                    return thunks

                for strip in range(NSTRIP):
                    q_sl = slice(strip * 512, (strip + 1) * 512)
                    at_t = at_ps.tile([128, 512], F32, tag="at",
                                      name="at_t")
                    sm_ps = sum_ps.tile([128, 512], F32, tag="sums",
                                        name="sm_ps")
                    es_tiles = []

                    def scores(p):
                        scp = sc_ps.tile([128, 1024], F32, tag="sc",
                                         name="scp")
                        for i in range(2):
                            kt = 2 * p + i
                            nc.tensor.matmul(
                                scp[:, i * 512:(i + 1) * 512],
                                kt_sb[:, kt * 128:(kt + 1) * 128],
                                qt_sb[:, q_sl],
                                start=True, stop=True,
                                skip_group_check=True)
                        es2 = esp.tile([128, 2, 512], FP8, tag="es",
                                       name="es2")
                        nc.scalar.activation(
                            es2[:].rearrange("p t n -> p (t n)"), scp[:],
                            mybir.ActivationFunctionType.Exp,
                            scale=float(INV_SQRT_HD))
                        if debug and b == 0 and strip == 0:
                            nc.sync.dma_start(
                                out=dbg_es[:, p * 1024:(p + 1) * 1024],
                                in_=es2[:].rearrange("p t n -> p (t n)"))
                        es_tiles.append(es2)

                    def attnsum(p):
                        es2 = es_tiles[p]
                        nc.tensor.matmul(
                            at_t[:], v_hi[:, 2 * p:2 * p + 2, :], es2[:],
                            start=(p == 0), stop=False,
                            perf_mode=DR, skip_group_check=True)
                        nc.tensor.matmul(
                            at_t[:], v_lo[:, 2 * p:2 * p + 2, :], es2[:],
                            start=False, stop=(p == NPAIR - 1),
                            perf_mode=DR, skip_group_check=True)
                        nc.tensor.matmul(
                            sm_ps[:], ones_dr[:], es2[:],
                            start=(p == 0), stop=(p == NPAIR - 1),
                            perf_mode=DR, skip_group_check=True)

                    for p in range(NPAIR):
                        scores(p)
                        drain(2 if len(pending) > 8 - p else 1)
                        if p >= 1:
                            attnsum(p - 1)
                    attnsum(NPAIR - 1)

                    # softmax denominators: [1,512] -> [128,4] columns ->
                    # 128-lane reciprocal; scaling rides the outproj copies
                    row_sm = smallp.tile([1, 512], F32, tag="rowsm",
                                         name="row_sm")
                    nc.vector.tensor_copy(row_sm[:], sm_ps[0:1, :])
                    rcol_sb = smallp.tile([128, NQT], F32, tag="rcol",
                                          name="rcol_sb")
                    for j in range(NQT):
                        nc.sync.dma_start(
                            out=rcol_sb[:, j:j + 1],
                            in_=row_sm[0:1, j * 128:(j + 1) * 128])
                    rc_sb = smallp.tile([128, NQT], F32, tag="rc",
                                        name="rc_sb")
                    nc.vector.reciprocal(rc_sb[:], rcol_sb[:])

                    atn_sb = smallp.tile([HD, 512], BF16, tag="atn",
                                         name="atn_sb")
                    nc.vector.tensor_copy(atn_sb[:], at_t[:])
                    if debug and b == 0 and strip == 0:
                        nc.sync.dma_start(out=dbg_sums[:], in_=row_sm[:])
                        nc.sync.dma_start(out=dbg_rc[:], in_=rc_sb[:])
                        nc.sync.dma_start(out=dbg_atn[:], in_=atn_sb[:])

                    drain(len(pending))
                    pending = outproj_thunks(strip, atn_sb, rc_sb)
                for t in pending:
                    t()

            # ---------- program ----------
            qk0 = emit_qk(0, halves=True)
            v0 = emit_v(0)
            if debug:
                nc.sync.dma_start(out=dbg_q[:], in_=qk0[0][:])
                nc.sync.dma_start(out=dbg_k[:], in_=qk0[1][:])
                nc.sync.dma_start(
                    out=dbg_vhi[:], in_=v0[0][:].rearrange("p t m -> p (t m)"))
                nc.sync.dma_start(
                    out=dbg_vlo[:], in_=v0[1][:].rearrange("p t m -> p (t m)"))
            emit_attention(0, *qk0, *v0, filler=lora_thunks())
            qk1 = emit_qk(1, halves=False)
            v1 = emit_v(1)
            emit_attention(1, *qk1, *v1, filler=[])

    nc.compile()
    _CACHE[key] = nc
    return nc


def _prep_in_maps(inputs):
    bf = ml_dtypes.bfloat16
    hidden = np.asarray(inputs["hidden_states"], dtype=np.float32)
    control = np.asarray(inputs["control_states"], dtype=np.float32)
    Wq = np.asarray(inputs["Wq"], dtype=np.float32)
    Wk = np.asarray(inputs["Wk"], dtype=np.float32)
    Wv = np.asarray(inputs["Wv"], dtype=np.float32)
    Wo = np.asarray(inputs["Wo"], dtype=np.float32)
    bo = np.asarray(inputs["bo"], dtype=np.float32)
    ld = np.asarray(inputs["lora_down"], dtype=np.float32)
    lu = np.asarray(inputs["lora_up"], dtype=np.float32)

    xT = np.ascontiguousarray(hidden.reshape(SG, D).T).astype(bf)
    cT_full = np.ascontiguousarray(control.reshape(SG, D).T).astype(bf)
    ldT = np.ascontiguousarray(ld.T).astype(bf)
    luT = np.ascontiguousarray(lu.T).astype(bf)
    bo_in = np.ascontiguousarray(bo.reshape(1, D))

    in_maps = []
    for c in range(N_CORES):
        hs = slice(c * HD, (c + 1) * HD)
        rs = slice(c * ROWS_PER_CORE, (c + 1) * ROWS_PER_CORE)
        in_maps.append({
            "xT": xT,
            "wqT": np.ascontiguousarray(Wq[hs, :].T).astype(bf),
            "wkT": np.ascontiguousarray(Wk[hs, :].T).astype(bf),
            "wvT": np.ascontiguousarray(Wv[hs, :].T).astype(bf),
            "woT": np.ascontiguousarray(Wo[:, hs].T).astype(bf),
            "cT": np.ascontiguousarray(cT_full[:, rs]),
            "ldT": ldT,
            "luT": luT,
            "bo": bo_in,
        })
    return in_maps


def _reduce_outputs(results):
    total = np.zeros((SG, D), dtype=np.float32)
    for c in range(N_CORES):
        total += results[c]["out"].astype(np.float32)
    for c in range(N_CORES):
        rs = slice(c * ROWS_PER_CORE, (c + 1) * ROWS_PER_CORE)
        total[rs] += results[c]["lora_out"].astype(np.float32)
    return total.reshape(B, S, D)


def kernel(**inputs):
    nc = build_program()
    in_maps = _prep_in_maps(inputs)
    res = run_bass_kernel_spmd(nc, in_maps, list(range(N_CORES)))
    return _reduce_outputs(res.results)
